# revision 13
# baseline (speedup 1.0000x reference)
"""DMFNS attention head (distance-based bistochastic attention) on 8 trn2 cores.

Sharding: 8 cores = 4 batches x 2 query-row halves. Each core computes its
[1024 x 2048] row block of the S x S kernel in transposed layout
(keys j on partitions, queries i on free axis), so the column sums N_C are a
free-axis reduce (fused into the Exp activation's accum_out). The cross-half
N_C sum is a 2-core AllReduce per batch pair.

Math notes:
  attn = exp(-max(||q_i-k_j||^2, eps)/BW); eps clamp is numerically irrelevant
  (sq ~ 1e3 >> 1e-12) and skipped.
  The N_R^-a row factor cancels in the final row normalization, so only
  N_C^-a is applied: probs^T[j,i] = attn^T[j,i]*c_j / sum_j'(attn*c_j'),
  c = N_C^-0.5.
  ||q-k||^2 is produced directly by the QK matmul via two augmented
  contraction rows (k-side [1, -k2/2], q-side [-q2/2, 1]), giving
  psum = qk - q2/2 - k2/2 = -sq/2; attn = Exp((2/BW)*psum).
  v's bias folds into the output: out = probs @ (x Wv^T) + bv since
  probs rows sum to 1.
"""

import os
import sys

import numpy as np

for _p in ("/opt/trn_rl_repo", "/root/.axon_site/_ro/trn_rl_repo"):
    if os.path.isdir(_p) and _p not in sys.path:
        sys.path.insert(0, _p)
        break

import concourse.bacc as bacc
import concourse.mybir as mybir
import concourse.tile as tile
from concourse import bass_utils

F32 = mybir.dt.float32
F32R = mybir.dt.float32r
BF16 = mybir.dt.bfloat16
AF = mybir.ActivationFunctionType
ALU = mybir.AluOpType

B, S, D = 4, 2048, 512
SH = S // 2          # query rows per core
NT = S // 128        # 16 key tiles
CT = D // 128        # 4 contraction tiles
NIC = SH // 512      # 2 query chunks of 512
BW = 512.0
SCALE_EXP = 2.0 / BW
A = 0.5

REPLICA_GROUPS = [[0, 1], [2, 3], [4, 5], [6, 7]]

_cache = {}


def _r(ap):
    return ap.bitcast(F32R)


def _body(nc, tc, xt, xtq, wqT, wkT, wvT, bq2, bk2, bv2, probsT, outT):
    # SBUF left stack (released mid-kernel, LIFO): keep -> x -> qk -> w
    # SBUF right stack (released at the end): attn -> v -> tail
    ps_mm = tc.alloc_tile_pool(name="ps_mm", bufs=4, space="PSUM")
    ps_vec = tc.alloc_tile_pool(name="ps_vec", bufs=2, space="PSUM")
    ps_av = tc.alloc_tile_pool(name="ps_av", bufs=2, space="PSUM")
    p_keep = tc.alloc_tile_pool(name="p_keep", bufs=1, side="left")
    p_x = tc.alloc_tile_pool(name="p_x", bufs=1, side="left")
    p_qk = tc.alloc_tile_pool(name="p_qk", bufs=1, side="left")
    p_w = tc.alloc_tile_pool(name="p_w", bufs=1, side="left")
    dram = tc.alloc_tile_pool(name="dram", bufs=1, space="DRAM")

    # ---- input loads (q/k weights first so projections can start early)
    wq_sb = [p_w.tile([128, D], BF16, name=f"wq{c}", tag=f"wq{c}") for c in range(CT)]
    xtq_sb = [p_w.tile([128, SH], BF16, name=f"xtq{c}", tag=f"xtq{c}") for c in range(CT)]
    wk_sb = [p_w.tile([128, D], BF16, name=f"wk{c}", tag=f"wk{c}") for c in range(CT)]
    xt_sb = [p_x.tile([128, S], BF16, name=f"xt{c}", tag=f"xt{c}") for c in range(CT)]
    wv_sb = [p_x.tile([128, D], BF16, name=f"wv{c}", tag=f"wv{c}") for c in range(CT)]
    bq_sb = p_keep.tile([128, CT], F32, name="bq_sb")
    bk_sb = p_keep.tile([128, CT], F32, name="bk_sb")
    bv_sb = p_keep.tile([128, CT], F32, name="bv_sb")
    for c in range(CT):
        nc.sync.dma_start(wq_sb[c], wqT[c * 128:(c + 1) * 128, :])
        nc.sync.dma_start(xtq_sb[c], xtq[c * 128:(c + 1) * 128, :])
    nc.sync.dma_start(bq_sb, bq2)
    for c in range(CT):
        nc.sync.dma_start(wk_sb[c], wkT[c * 128:(c + 1) * 128, :])
    nc.sync.dma_start(bk_sb, bk2)
    for c in range(CT):
        nc.sync.dma_start(xt_sb[c], xt[c * 128:(c + 1) * 128, :])
        nc.sync.dma_start(wv_sb[c], wvT[c * 128:(c + 1) * 128, :])
    nc.sync.dma_start(bv_sb, bv2)

    ones = p_keep.tile([128, 1], BF16, name="ones")
    nc.vector.memset(ones, 1.0)
    onesr = p_keep.tile([1, 128], F32R, name="onesr")
    nc.vector.memset(onesr.bitcast(mybir.dt.uint32), 0x3F800000)

    # ---- q^T projection: qT[d,i] = Wq @ x^T + bq (bias per-partition d)
    qT = [p_qk.tile([128, SH], BF16, name=f"qT{d}", tag=f"qT{d}") for d in range(CT)]
    for dt in range(CT):
        for ic in range(NIC):
            ps = ps_mm.tile([128, 512], F32, name=f"psq{dt}_{ic}", tag="mm", bufs=4)
            for ct in range(CT):
                nc.tensor.matmul(
                    ps,
                    wq_sb[ct][:, dt * 128:(dt + 1) * 128],
                    xtq_sb[ct][:, ic * 512:(ic + 1) * 512],
                    start=(ct == 0), stop=(ct == CT - 1))
            nc.scalar.activation(
                qT[dt][:, ic * 512:(ic + 1) * 512], ps, AF.Identity,
                bias=bq_sb[:, dt:dt + 1], scale=1.0)

    # ---- q2[i] = sum_d qT[d,i]^2 -> qaug rows [-q2/2; 1]
    qaug = p_qk.tile([2, SH], BF16, name="qaug")
    nc.vector.memset(qaug, 1.0)  # row1 stays 1.0; row0 overwritten below
    q2ps = [ps_vec.tile([1, 512], F32, name=f"q2ps{ic}", tag="vec", bufs=2)
            for ic in range(NIC)]
    for dt in range(CT):
        qsq = p_qk.tile([128, SH], BF16, name=f"qsq{dt}", tag="qsq", bufs=2)
        nc.scalar.activation(qsq, qT[dt], AF.Square)
        for ic in range(NIC):
            nc.tensor.matmul(
                q2ps[ic], ones, qsq[:, ic * 512:(ic + 1) * 512],
                start=(dt == 0), stop=(dt == CT - 1))
    for ic in range(NIC):
        nc.scalar.activation(qaug[0:1, ic * 512:(ic + 1) * 512], q2ps[ic],
                             AF.Copy, scale=-0.5)

    # ---- k^T projection (full batch): kT[d,j]
    kT = [p_qk.tile([128, S], BF16, name=f"kT{d}", tag=f"kT{d}") for d in range(CT)]
    for dt in range(CT):
        for jc in range(S // 512):
            ps = ps_mm.tile([128, 512], F32, name=f"psk{dt}_{jc}", tag="mm", bufs=4)
            for ct in range(CT):
                nc.tensor.matmul(
                    ps,
                    wk_sb[ct][:, dt * 128:(dt + 1) * 128],
                    xt_sb[ct][:, jc * 512:(jc + 1) * 512],
                    start=(ct == 0), stop=(ct == CT - 1))
            nc.scalar.activation(
                kT[dt][:, jc * 512:(jc + 1) * 512], ps, AF.Identity,
                bias=bk_sb[:, dt:dt + 1], scale=1.0)

    # ---- k2[j] -> kaug rows [1; -k2/2]
    kaug = p_qk.tile([2, S], BF16, name="kaug")
    nc.vector.memset(kaug, 1.0)  # row0 stays 1.0; row1 filled via DMA below
    kneg = p_qk.tile([1, S], BF16, name="kneg")
    ksq = [p_qk.tile([128, S], BF16, name=f"ksq{d}", tag=f"ksq{d}")
           for d in range(CT)]
    for dt in range(CT):
        nc.scalar.activation(ksq[dt], kT[dt], AF.Square)
    for jc in range(S // 512):
        k2ps = ps_vec.tile([1, 512], F32, name=f"k2ps{jc}", tag="vec", bufs=2)
        for dt in range(CT):
            nc.tensor.matmul(
                k2ps, ones, ksq[dt][:, jc * 512:(jc + 1) * 512],
                start=(dt == 0), stop=(dt == CT - 1))
        nc.scalar.activation(kneg[0:1, jc * 512:(jc + 1) * 512], k2ps,
                             AF.Copy, scale=-0.5)
        # DMA can write partition 1 (engines cannot address base partition 1)
        nc.sync.dma_start(kaug[1:2, jc * 512:(jc + 1) * 512],
                          kneg[0:1, jc * 512:(jc + 1) * 512])

    p_w.release()
    p_attn = tc.alloc_tile_pool(name="p_attn", bufs=1, side="right")

    # ---- scores^T + exp, with fused per-tile column-sum partials
    attn = [p_attn.tile([128, SH], F32R, name=f"attn{t}", tag=f"attn{t}")
            for t in range(NT)]
    ncp0 = p_keep.tile([128, NT], F32, name="ncp0")
    ncp1 = p_keep.tile([128, NT], F32, name="ncp1")
    ncs = p_keep.tile([128, NT], F32, name="ncs")
    nct = p_keep.tile([128, NT], F32, name="nct")
    cc_in = dram.tile([128, NT], F32, name="cc_in")
    cc_out = dram.tile([128, NT], F32, name="cc_out")
    for t in range(NT):
        for ic in range(NIC):
            ps = ps_mm.tile([128, 512], F32, name=f"pss{t}_{ic}", tag="mm", bufs=4)
            for ct in range(CT):
                nc.tensor.matmul(
                    ps,
                    kT[ct][:, t * 128:(t + 1) * 128],
                    qT[ct][:, ic * 512:(ic + 1) * 512],
                    start=(ct == 0), stop=False)
            nc.tensor.matmul(
                ps,
                kaug[:, t * 128:(t + 1) * 128],
                qaug[:, ic * 512:(ic + 1) * 512],
                start=False, stop=True)
            acc = (ncp0 if ic == 0 else ncp1)[:, t:t + 1]
            nc.scalar.activation(
                attn[t][:, ic * 512:(ic + 1) * 512], ps, AF.Exp,
                scale=SCALE_EXP, accum_out=acc)
    nc.vector.tensor_add(ncs, ncp0, ncp1)
    nc.sync.dma_start(cc_in, ncs)
    nc.gpsimd.collective_compute(
        "AllReduce", ALU.add, replica_groups=REPLICA_GROUPS,
        ins=[cc_in.opt()], outs=[cc_out.opt()])
    nc.sync.dma_start(nct, cc_out)

    p_qk.release()

    # ---- v projection (overlaps the collective): v[j,dd] = x Wv^T (no bias)
    p_v = tc.alloc_tile_pool(name="p_v", bufs=1, side="right")
    v_sb = [p_v.tile([128, D], BF16, name=f"v{t}", tag=f"v{t}") for t in range(NT)]
    for t in range(NT):
        ps = ps_mm.tile([128, 512], F32, name=f"psv{t}", tag="mm", bufs=4)
        for ct in range(CT):
            nc.tensor.matmul(
                ps,
                xt_sb[ct][:, t * 128:(t + 1) * 128],
                wv_sb[ct],
                start=(ct == 0), stop=(ct == CT - 1))
        nc.vector.tensor_copy(v_sb[t], ps)

    p_x.release()
    p_tail = tc.alloc_tile_pool(name="p_tail", bufs=1, side="right")

    # ---- c_j = N_C^-0.5 on DVE only (avoids ACT table-set switches):
    # rsqrt bit-hack seed + 2 Newton iterations, all on [128, NT]
    I32 = mybir.dt.int32
    c_sb = p_keep.tile([128, NT], F32, name="c_sb")
    cw0 = p_keep.tile([128, NT], F32, name="cw0")
    cw1 = p_keep.tile([128, NT], F32, name="cw1")
    nc.vector.tensor_scalar(cw0.bitcast(I32), nct.bitcast(I32), 1, None,
                            ALU.arith_shift_right)
    nc.vector.tensor_scalar(c_sb.bitcast(I32), cw0.bitcast(I32), 0x5F3759DF,
                            -1, ALU.subtract, ALU.mult)
    for _ in range(2):
        nc.vector.tensor_tensor(cw0, c_sb, c_sb, ALU.mult)          # y^2
        nc.vector.tensor_tensor(cw1, nct, cw0, ALU.mult)            # x y^2
        nc.vector.tensor_scalar(cw1, cw1, -0.5, 1.5, ALU.mult, ALU.add)
        nc.vector.tensor_tensor(c_sb, c_sb, cw1, ALU.mult)          # y *= ...
    # ---- T' = attn*c (bf16) and r[i] = sum_j T'[j,i]
    tp = [p_v.tile([128, SH], BF16, name=f"tp{t}", tag=f"tp{t}")
          for t in range(NT)]
    rps = [ps_vec.tile([1, 512], F32, name=f"rps{ic}", tag="vec", bufs=2)
           for ic in range(NIC)]
    for t in range(NT):
        nc.vector.tensor_scalar(tp[t], attn[t].bitcast(F32),
                                c_sb[:, t:t + 1], None, ALU.mult)
    for t in range(NT):
        for ic in range(NIC):
            nc.tensor.matmul(
                rps[ic], ones,
                tp[t][:, ic * 512:(ic + 1) * 512],
                start=(t == 0), stop=(t == NT - 1))

    # ---- bc[p,i] = 1/r_i: r row -> PE outer-product broadcast -> DVE recip
    rrow = p_tail.tile([1, SH], F32R, name="rrow")
    bcr = p_tail.tile([128, SH], F32, name="bcr")
    bcs = p_tail.tile([128, SH], F32, name="bcs")
    bc = p_tail.tile([128, SH], F32, name="bc")
    for ic in range(NIC):
        nc.scalar.activation(rrow[0:1, ic * 512:(ic + 1) * 512], rps[ic],
                             AF.Copy)
        bps = ps_mm.tile([128, 512], F32, name=f"bps{ic}", tag="mm", bufs=4)
        nc.tensor.matmul(bps, onesr,
                         rrow[0:1, ic * 512:(ic + 1) * 512],
                         start=True, stop=True)
        nc.scalar.activation(bcr[:, ic * 512:(ic + 1) * 512], bps, AF.Copy)
    nc.vector.reciprocal_approx_accurate(bc, bcr, bcs)

    # ---- probs^T = (attn * c_j) * (1/r_i) -> fp32 rotating buffers -> DMA
    for t in range(NT):
        pbuf = p_tail.tile([128, SH], F32, name=f"pbuf{t}", tag="pbuf", bufs=3)
        nc.vector.scalar_tensor_tensor(
            pbuf, attn[t].bitcast(F32), c_sb[:, t:t + 1], bc,
            ALU.mult, ALU.mult)
        nc.sync.dma_start(probsT[t * 128:(t + 1) * 128, :], pbuf)

    # ---- out^T = ((v^T @ T'^T) * rinv_i) + bv
    for dt in range(CT):
        for ic in range(NIC):
            aps = ps_av.tile([128, 512], F32, name=f"av{dt}_{ic}", tag="av", bufs=2)
            for t in range(NT):
                nc.tensor.matmul(
                    aps,
                    v_sb[t][:, dt * 128:(dt + 1) * 128],
                    tp[t][:, ic * 512:(ic + 1) * 512],
                    start=(t == 0), stop=(t == NT - 1))
            osb = p_tail.tile([128, 512], F32, name=f"osb{dt}_{ic}", tag="osb", bufs=2)
            nc.vector.tensor_tensor(osb, aps,
                                    bc[:, ic * 512:(ic + 1) * 512], ALU.mult)
            nc.scalar.activation(osb, osb, AF.Identity,
                                 bias=bv_sb[:, dt:dt + 1], scale=1.0)
            nc.sync.dma_start(
                outT[dt * 128:(dt + 1) * 128, ic * 512:(ic + 1) * 512], osb)

    ps_av.release()
    ps_vec.release()
    ps_mm.release()
    dram.release()
    p_tail.release()
    p_v.release()
    p_attn.release()
    p_keep.release()


def _build():
    nc = bacc.Bacc("TRN2", target_bir_lowering=False, debug=False,
                   enable_asserts=False, num_devices=8)
    xt = nc.dram_tensor("xt", [D, S], BF16, kind="ExternalInput").ap()
    xtq = nc.dram_tensor("xtq", [D, SH], BF16, kind="ExternalInput").ap()
    wqT = nc.dram_tensor("wqT", [D, D], BF16, kind="ExternalInput").ap()
    wkT = nc.dram_tensor("wkT", [D, D], BF16, kind="ExternalInput").ap()
    wvT = nc.dram_tensor("wvT", [D, D], BF16, kind="ExternalInput").ap()
    bq2 = nc.dram_tensor("bq2", [128, CT], F32, kind="ExternalInput").ap()
    bk2 = nc.dram_tensor("bk2", [128, CT], F32, kind="ExternalInput").ap()
    bv2 = nc.dram_tensor("bv2", [128, CT], F32, kind="ExternalInput").ap()
    probsT = nc.dram_tensor("probsT", [S, SH], F32, kind="ExternalOutput").ap()
    outT = nc.dram_tensor("outT", [D, SH], F32, kind="ExternalOutput").ap()

    with tile.TileContext(nc) as tc:
        _body(nc, tc, xt, xtq, wqT, wkT, wvT, bq2, bk2, bv2, probsT, outT)
    nc.compile()
    return nc


def _get_nc():
    if "nc" not in _cache:
        _cache["nc"] = _build()
    return _cache["nc"]


def _in_maps(x, Wq, bq, Wk, bk, Wv, bv):
    import ml_dtypes
    bf16 = ml_dtypes.bfloat16
    WqT = np.ascontiguousarray(Wq.T).astype(bf16)
    WkT = np.ascontiguousarray(Wk.T).astype(bf16)
    WvT = np.ascontiguousarray(Wv.T).astype(bf16)
    bq2 = np.ascontiguousarray(bq.reshape(CT, 128).T, dtype=np.float32)
    bk2 = np.ascontiguousarray(bk.reshape(CT, 128).T, dtype=np.float32)
    bv2 = np.ascontiguousarray(bv.reshape(CT, 128).T, dtype=np.float32)
    maps = []
    for core in range(8):
        b, h = core // 2, core % 2
        xtb = np.ascontiguousarray(x[b].T).astype(bf16)
        maps.append({
            "xt": xtb,
            "xtq": np.ascontiguousarray(xtb[:, h * SH:(h + 1) * SH]),
            "wqT": WqT, "wkT": WkT, "wvT": WvT,
            "bq2": bq2, "bk2": bk2, "bv2": bv2,
        })
    return maps


def run(x, Wq, bq, Wk, bk, Wv, bv, trace=False):
    nc = _get_nc()
    x = np.asarray(x, dtype=np.float32)
    maps = _in_maps(x, np.asarray(Wq), np.asarray(bq), np.asarray(Wk),
                    np.asarray(bk), np.asarray(Wv), np.asarray(bv))
    bkr = bass_utils.run_bass_kernel_spmd(nc, maps, core_ids=list(range(8)),
                                          trace=trace)
    out = np.empty((B, 1, S, D), dtype=np.float32)
    probs = np.empty((B, 1, S, S), dtype=np.float32)
    for core in range(8):
        b, h = core // 2, core % 2
        res = bkr.results[core]
        probs[b, 0, h * SH:(h + 1) * SH, :] = np.asarray(res["probsT"]).T
        out[b, 0, h * SH:(h + 1) * SH, :] = np.asarray(res["outT"]).T
    return (out, probs), bkr


def kernel(x, Wq, bq, Wk, bk, Wv, bv):
    (out, probs), _ = run(x, Wq, bq, Wk, bk, Wv, bv, trace=False)
    return out, probs


# revision 15
# speedup vs baseline: 1.1109x; 1.1109x over previous
"""DMFNS attention head (distance-based bistochastic attention) on 8 trn2 cores.

Sharding: 8 cores = 4 batches x 2 query-row halves. Each core computes its
[1024 x 2048] row block of the S x S kernel in transposed layout
(keys j on partitions, queries i on free axis), so the column sums N_C are a
free-axis reduce (fused into the Exp activation's accum_out). The cross-half
N_C sum is a 2-core AllReduce per batch pair.

Math notes:
  attn = exp(-max(||q_i-k_j||^2, eps)/BW); eps clamp is numerically irrelevant
  (sq ~ 1e3 >> 1e-12) and skipped.
  The N_R^-a row factor cancels in the final row normalization, so only
  N_C^-a is applied: probs^T[j,i] = attn^T[j,i]*c_j / sum_j'(attn*c_j'),
  c = N_C^-0.5.
  ||q-k||^2 is produced directly by the QK matmul via two augmented
  contraction rows (k-side [1, -k2/2], q-side [-q2/2, 1]), giving
  psum = qk - q2/2 - k2/2 = -sq/2; attn = Exp((2/BW)*psum).
  v's bias folds into the output: out = probs @ (x Wv^T) + bv since
  probs rows sum to 1.
"""

import os
import sys

import numpy as np

for _p in ("/opt/trn_rl_repo", "/root/.axon_site/_ro/trn_rl_repo"):
    if os.path.isdir(_p) and _p not in sys.path:
        sys.path.insert(0, _p)
        break

import concourse.bacc as bacc
import concourse.mybir as mybir
import concourse.tile as tile
from concourse.tile import add_dep_helper
from concourse import bass_utils

F32 = mybir.dt.float32
F32R = mybir.dt.float32r
BF16 = mybir.dt.bfloat16
AF = mybir.ActivationFunctionType
ALU = mybir.AluOpType

B, S, D = 4, 2048, 512
SH = S // 2          # query rows per core
NT = S // 128        # 16 key tiles
CT = D // 128        # 4 contraction tiles
NIC = SH // 512      # 2 query chunks of 512
BW = 512.0
SCALE_EXP = 2.0 / BW
A = 0.5

REPLICA_GROUPS = [[0, 1], [2, 3], [4, 5], [6, 7]]

_cache = {}


def _r(ap):
    return ap.bitcast(F32R)


def _body(nc, tc, xt, xtq, wqT, wkT, wvT, bq2, bk2, bv2, probsT, outT):
    # SBUF left stack (released mid-kernel, LIFO): keep -> x -> qk -> w
    # SBUF right stack (released at the end): attn -> v -> tail
    ps_mm = tc.alloc_tile_pool(name="ps_mm", bufs=3, space="PSUM")
    ps_vec = tc.alloc_tile_pool(name="ps_vec", bufs=2, space="PSUM")
    ps_av = tc.alloc_tile_pool(name="ps_av", bufs=3, space="PSUM")
    p_keep = tc.alloc_tile_pool(name="p_keep", bufs=1, side="left")
    p_x = tc.alloc_tile_pool(name="p_x", bufs=1, side="left")
    p_qk = tc.alloc_tile_pool(name="p_qk", bufs=1, side="left")
    p_w = tc.alloc_tile_pool(name="p_w", bufs=1, side="left")
    dram = tc.alloc_tile_pool(name="dram", bufs=1, space="DRAM")

    # ---- input loads (q/k weights first so projections can start early)
    wq_sb = [p_w.tile([128, D], BF16, name=f"wq{c}", tag=f"wq{c}") for c in range(CT)]
    xtq_sb = [p_w.tile([128, SH], BF16, name=f"xtq{c}", tag=f"xtq{c}") for c in range(CT)]
    wk_sb = [p_w.tile([128, D], BF16, name=f"wk{c}", tag=f"wk{c}") for c in range(CT)]
    xt_sb = [p_x.tile([128, S], BF16, name=f"xt{c}", tag=f"xt{c}") for c in range(CT)]
    wv_sb = [p_x.tile([128, D], BF16, name=f"wv{c}", tag=f"wv{c}") for c in range(CT)]
    bq_sb = p_keep.tile([128, CT], F32, name="bq_sb")
    bk_sb = p_keep.tile([128, CT], F32, name="bk_sb")
    bv_sb = p_keep.tile([128, CT], F32, name="bv_sb")
    for c in range(CT):
        nc.sync.dma_start(wq_sb[c], wqT[c * 128:(c + 1) * 128, :])
        nc.sync.dma_start(xtq_sb[c], xtq[c * 128:(c + 1) * 128, :])
    nc.sync.dma_start(bq_sb, bq2)
    for c in range(CT):
        nc.sync.dma_start(wk_sb[c], wkT[c * 128:(c + 1) * 128, :])
    nc.sync.dma_start(bk_sb, bk2)
    for c in range(CT):
        nc.sync.dma_start(xt_sb[c], xt[c * 128:(c + 1) * 128, :])
        nc.sync.dma_start(wv_sb[c], wvT[c * 128:(c + 1) * 128, :])
    nc.sync.dma_start(bv_sb, bv2)

    ones = p_keep.tile([128, 1], BF16, name="ones")
    nc.vector.memset(ones, 1.0)
    onesr = p_keep.tile([1, 128], F32R, name="onesr")
    nc.vector.memset(onesr.bitcast(mybir.dt.uint32), 0x3F800000)

    # ---- q^T projection: qT[d,i] = Wq @ x^T + bq (bias per-partition d)
    qT = [p_qk.tile([128, SH], BF16, name=f"qT{d}", tag=f"qT{d}") for d in range(CT)]
    for dt in range(CT):
        for ic in range(NIC):
            ps = ps_mm.tile([128, 512], F32, name=f"psq{dt}_{ic}", tag="mm", bufs=3)
            for ct in range(CT):
                nc.tensor.matmul(
                    ps,
                    wq_sb[ct][:, dt * 128:(dt + 1) * 128],
                    xtq_sb[ct][:, ic * 512:(ic + 1) * 512],
                    start=(ct == 0), stop=(ct == CT - 1))
            nc.scalar.activation(
                qT[dt][:, ic * 512:(ic + 1) * 512], ps, AF.Identity,
                bias=bq_sb[:, dt:dt + 1], scale=1.0)

    # ---- q2[i] = sum_d qT[d,i]^2 -> qaug rows [-q2/2; 1]
    qaug = p_qk.tile([2, SH], BF16, name="qaug")
    nc.vector.memset(qaug, 1.0)  # row1 stays 1.0; row0 overwritten below
    q2ps = [ps_vec.tile([1, 512], F32, name=f"q2ps{ic}", tag="vec", bufs=2)
            for ic in range(NIC)]
    for dt in range(CT):
        qsq = p_qk.tile([128, SH], BF16, name=f"qsq{dt}", tag="qsq", bufs=2)
        nc.scalar.activation(qsq, qT[dt], AF.Square)
        for ic in range(NIC):
            nc.tensor.matmul(
                q2ps[ic], ones, qsq[:, ic * 512:(ic + 1) * 512],
                start=(dt == 0), stop=(dt == CT - 1))
    for ic in range(NIC):
        nc.scalar.activation(qaug[0:1, ic * 512:(ic + 1) * 512], q2ps[ic],
                             AF.Copy, scale=-0.5)

    # ---- k^T projection (full batch): kT[d,j]
    kT = [p_qk.tile([128, S], BF16, name=f"kT{d}", tag=f"kT{d}") for d in range(CT)]
    for dt in range(CT):
        for jc in range(S // 512):
            ps = ps_mm.tile([128, 512], F32, name=f"psk{dt}_{jc}", tag="mm", bufs=3)
            for ct in range(CT):
                nc.tensor.matmul(
                    ps,
                    wk_sb[ct][:, dt * 128:(dt + 1) * 128],
                    xt_sb[ct][:, jc * 512:(jc + 1) * 512],
                    start=(ct == 0), stop=(ct == CT - 1))
            nc.scalar.activation(
                kT[dt][:, jc * 512:(jc + 1) * 512], ps, AF.Identity,
                bias=bk_sb[:, dt:dt + 1], scale=1.0)

    # ---- k2[j] -> kaug rows [1; -k2/2]
    kaug = p_qk.tile([2, S], BF16, name="kaug")
    nc.vector.memset(kaug, 1.0)  # row0 stays 1.0; row1 filled via DMA below
    kneg = p_qk.tile([1, S], BF16, name="kneg")
    ksq = [p_qk.tile([128, S], BF16, name=f"ksq{d}", tag=f"ksq{d}")
           for d in range(CT)]
    for dt in range(CT):
        nc.scalar.activation(ksq[dt], kT[dt], AF.Square)
    for jc in range(S // 512):
        k2ps = ps_vec.tile([1, 512], F32, name=f"k2ps{jc}", tag="vec", bufs=2)
        for dt in range(CT):
            nc.tensor.matmul(
                k2ps, ones, ksq[dt][:, jc * 512:(jc + 1) * 512],
                start=(dt == 0), stop=(dt == CT - 1))
        nc.scalar.activation(kneg[0:1, jc * 512:(jc + 1) * 512], k2ps,
                             AF.Copy, scale=-0.5)
        # DMA can write partition 1 (engines cannot address base partition 1)
        nc.sync.dma_start(kaug[1:2, jc * 512:(jc + 1) * 512],
                          kneg[0:1, jc * 512:(jc + 1) * 512])

    p_w.release()
    p_attn = tc.alloc_tile_pool(name="p_attn", bufs=1, side="right")

    # ---- scores^T + exp, with fused per-tile column-sum partials
    attn = [p_attn.tile([128, SH], F32R, name=f"attn{t}", tag=f"attn{t}")
            for t in range(NT)]
    ncp0 = p_keep.tile([128, NT], F32, name="ncp0")
    ncp1 = p_keep.tile([128, NT], F32, name="ncp1")
    ncs = p_keep.tile([128, NT], F32, name="ncs")
    nct = p_keep.tile([128, NT], F32, name="nct")
    cc_in = dram.tile([128, NT], F32, name="cc_in")
    cc_out = dram.tile([128, NT], F32, name="cc_out")
    for t in range(NT):
        for ic in range(NIC):
            ps = ps_mm.tile([128, 512], F32, name=f"pss{t}_{ic}", tag="mm", bufs=3)
            for ct in range(CT):
                nc.tensor.matmul(
                    ps,
                    kT[ct][:, t * 128:(t + 1) * 128],
                    qT[ct][:, ic * 512:(ic + 1) * 512],
                    start=(ct == 0), stop=False)
            nc.tensor.matmul(
                ps,
                kaug[:, t * 128:(t + 1) * 128],
                qaug[:, ic * 512:(ic + 1) * 512],
                start=False, stop=True)
            acc = (ncp0 if ic == 0 else ncp1)[:, t:t + 1]
            nc.scalar.activation(
                attn[t][:, ic * 512:(ic + 1) * 512], ps, AF.Exp,
                scale=SCALE_EXP, accum_out=acc)
    nc.vector.tensor_add(ncs, ncp0, ncp1)
    ncs_dma = nc.sync.dma_start(cc_in, ncs)
    nc.gpsimd.collective_compute(
        "AllReduce", ALU.add, replica_groups=REPLICA_GROUPS,
        ins=[cc_in.opt()], outs=[cc_out.opt()])
    nc.sync.dma_start(nct, cc_out)

    p_qk.release()

    # ---- v projection (overlaps the collective): v[j,dd] = x Wv^T (no bias)
    p_v = tc.alloc_tile_pool(name="p_v", bufs=1, side="right")
    v_sb = [p_v.tile([128, D], BF16, name=f"v{t}", tag=f"v{t}") for t in range(NT)]
    for t in range(NT):
        ps = ps_mm.tile([128, 512], F32, name=f"psv{t}", tag="mm", bufs=3)
        for ct in range(CT):
            mm = nc.tensor.matmul(
                ps,
                xt_sb[ct][:, t * 128:(t + 1) * 128],
                wv_sb[ct],
                start=(ct == 0), stop=(ct == CT - 1))
            if ct == 0:
                add_dep_helper(mm.ins, ncs_dma.ins, sync=False,
                               reason="keep v-proj in the collective window")
        nc.vector.tensor_copy(v_sb[t], ps)

    p_x.release()
    p_tail = tc.alloc_tile_pool(name="p_tail", bufs=1, side="right")

    # ---- c_j = N_C^-0.5 on DVE only (avoids ACT table-set switches):
    # rsqrt bit-hack seed + 2 Newton iterations, all on [128, NT]
    I32 = mybir.dt.int32
    c_sb = p_keep.tile([128, NT], F32R, name="c_sb")
    cy = p_keep.tile([128, NT], F32, name="cy")
    cw0 = p_keep.tile([128, NT], F32, name="cw0")
    cw1 = p_keep.tile([128, NT], F32, name="cw1")
    nc.vector.tensor_scalar(cw0.bitcast(I32), nct.bitcast(I32), 1, None,
                            ALU.arith_shift_right)
    nc.vector.tensor_scalar(cy.bitcast(I32), cw0.bitcast(I32), 0x5F3759DF,
                            -1, ALU.subtract, ALU.mult)
    for it in range(2):
        nc.vector.tensor_tensor(cw0, cy, cy, ALU.mult)              # y^2
        nc.vector.tensor_tensor(cw1, nct, cw0, ALU.mult)            # x y^2
        nc.vector.tensor_scalar(cw1, cw1, -0.5, 1.5, ALU.mult, ALU.add)
        # final iteration writes the f32r tile consumed by the r-matmul
        nc.vector.tensor_tensor(c_sb if it == 1 else cy,
                                cy, cw1, ALU.mult)                  # y *= ...
    # ---- T' = attn*c (bf16) and r[i] = sum_j T'[j,i]
    tp = [p_v.tile([128, SH], BF16, name=f"tp{t}", tag=f"tp{t}")
          for t in range(NT)]
    rps = [ps_vec.tile([1, 512], F32, name=f"rps{ic}", tag="vec", bufs=2)
           for ic in range(NIC)]
    for t in range(NT):
        for ic in range(NIC):
            nc.tensor.matmul(
                rps[ic], c_sb[:, t:t + 1],
                attn[t][:, ic * 512:(ic + 1) * 512],
                start=(t == 0), stop=(t == NT - 1))
    for t in range(NT):
        nc.vector.tensor_scalar(tp[t], attn[t].bitcast(F32),
                                c_sb[:, t:t + 1].bitcast(F32), None, ALU.mult)

    # ---- bc[p,i] = 1/r_i: r row -> PE outer-product broadcast -> DVE recip
    rrow = p_tail.tile([1, SH], F32R, name="rrow")
    bcr = p_tail.tile([128, SH], F32, name="bcr")
    bcs = p_tail.tile([128, SH], F32, name="bcs")
    bc = p_tail.tile([128, SH], F32, name="bc")
    for ic in range(NIC):
        nc.scalar.activation(rrow[0:1, ic * 512:(ic + 1) * 512], rps[ic],
                             AF.Copy)
        bps = ps_mm.tile([128, 512], F32, name=f"bps{ic}", tag="mm", bufs=3)
        nc.tensor.matmul(bps, onesr,
                         rrow[0:1, ic * 512:(ic + 1) * 512],
                         start=True, stop=True)
        nc.scalar.activation(bcr[:, ic * 512:(ic + 1) * 512], bps, AF.Copy)
    nc.vector.reciprocal_approx_accurate(bc, bcr, bcs)

    # ---- probs^T = (attn * c_j) * (1/r_i), interleaved with the AV groups
    # so the av psum slots recycle while probs STTs stream on DVE
    def av_group(g):
        dt, ic = divmod(g, NIC)
        aps = ps_av.tile([128, 512], F32, name=f"av{dt}_{ic}", tag="av", bufs=3)
        for t in range(NT):
            nc.tensor.matmul(
                aps,
                v_sb[t][:, dt * 128:(dt + 1) * 128],
                tp[t][:, ic * 512:(ic + 1) * 512],
                start=(t == 0), stop=(t == NT - 1))
        osb = p_tail.tile([128, 512], F32, name=f"osb{dt}_{ic}", tag="osb", bufs=2)
        nc.vector.tensor_tensor(osb, aps,
                                bc[:, ic * 512:(ic + 1) * 512], ALU.mult)
        nc.scalar.activation(osb, osb, AF.Identity,
                             bias=bv_sb[:, dt:dt + 1], scale=1.0)
        nc.sync.dma_start(
            outT[dt * 128:(dt + 1) * 128, ic * 512:(ic + 1) * 512], osb)

    for t in range(NT):
        pbuf = p_tail.tile([128, SH], F32, name=f"pbuf{t}", tag="pbuf", bufs=3)
        nc.vector.scalar_tensor_tensor(
            pbuf, attn[t].bitcast(F32), c_sb[:, t:t + 1].bitcast(F32), bc,
            ALU.mult, ALU.mult)
        nc.sync.dma_start(probsT[t * 128:(t + 1) * 128, :], pbuf)
        if t % 2 == 1:
            av_group(t // 2)

    ps_av.release()
    ps_vec.release()
    ps_mm.release()
    dram.release()
    p_tail.release()
    p_v.release()
    p_attn.release()
    p_keep.release()


def _build():
    nc = bacc.Bacc("TRN2", target_bir_lowering=False, debug=False,
                   enable_asserts=False, num_devices=8)
    xt = nc.dram_tensor("xt", [D, S], BF16, kind="ExternalInput").ap()
    xtq = nc.dram_tensor("xtq", [D, SH], BF16, kind="ExternalInput").ap()
    wqT = nc.dram_tensor("wqT", [D, D], BF16, kind="ExternalInput").ap()
    wkT = nc.dram_tensor("wkT", [D, D], BF16, kind="ExternalInput").ap()
    wvT = nc.dram_tensor("wvT", [D, D], BF16, kind="ExternalInput").ap()
    bq2 = nc.dram_tensor("bq2", [128, CT], F32, kind="ExternalInput").ap()
    bk2 = nc.dram_tensor("bk2", [128, CT], F32, kind="ExternalInput").ap()
    bv2 = nc.dram_tensor("bv2", [128, CT], F32, kind="ExternalInput").ap()
    probsT = nc.dram_tensor("probsT", [S, SH], F32, kind="ExternalOutput").ap()
    outT = nc.dram_tensor("outT", [D, SH], F32, kind="ExternalOutput").ap()

    with tile.TileContext(nc) as tc:
        _body(nc, tc, xt, xtq, wqT, wkT, wvT, bq2, bk2, bv2, probsT, outT)
    nc.compile()
    return nc


def _get_nc():
    if "nc" not in _cache:
        _cache["nc"] = _build()
    return _cache["nc"]


def _in_maps(x, Wq, bq, Wk, bk, Wv, bv):
    import ml_dtypes
    bf16 = ml_dtypes.bfloat16
    WqT = np.ascontiguousarray(Wq.T).astype(bf16)
    WkT = np.ascontiguousarray(Wk.T).astype(bf16)
    WvT = np.ascontiguousarray(Wv.T).astype(bf16)
    bq2 = np.ascontiguousarray(bq.reshape(CT, 128).T, dtype=np.float32)
    bk2 = np.ascontiguousarray(bk.reshape(CT, 128).T, dtype=np.float32)
    bv2 = np.ascontiguousarray(bv.reshape(CT, 128).T, dtype=np.float32)
    maps = []
    for core in range(8):
        b, h = core // 2, core % 2
        xtb = np.ascontiguousarray(x[b].T).astype(bf16)
        maps.append({
            "xt": xtb,
            "xtq": np.ascontiguousarray(xtb[:, h * SH:(h + 1) * SH]),
            "wqT": WqT, "wkT": WkT, "wvT": WvT,
            "bq2": bq2, "bk2": bk2, "bv2": bv2,
        })
    return maps


def run(x, Wq, bq, Wk, bk, Wv, bv, trace=False):
    nc = _get_nc()
    x = np.asarray(x, dtype=np.float32)
    maps = _in_maps(x, np.asarray(Wq), np.asarray(bq), np.asarray(Wk),
                    np.asarray(bk), np.asarray(Wv), np.asarray(bv))
    bkr = bass_utils.run_bass_kernel_spmd(nc, maps, core_ids=list(range(8)),
                                          trace=trace)
    out = np.empty((B, 1, S, D), dtype=np.float32)
    probs = np.empty((B, 1, S, S), dtype=np.float32)
    for core in range(8):
        b, h = core // 2, core % 2
        res = bkr.results[core]
        probs[b, 0, h * SH:(h + 1) * SH, :] = np.asarray(res["probsT"]).T
        out[b, 0, h * SH:(h + 1) * SH, :] = np.asarray(res["outT"]).T
    return (out, probs), bkr


def kernel(x, Wq, bq, Wk, bk, Wv, bv):
    (out, probs), _ = run(x, Wq, bq, Wk, bk, Wv, bv, trace=False)
    return out, probs
